# revision 42
# baseline (speedup 1.0000x reference)
"""LRNetLinear forward on 8 Trainium2 NeuronCores (tensor-parallel over out_features).

Math (per reference):
  3-way softmax over logits [theta_neg, 0, theta_pos]:
    en = exp(tn); ep = exp(tp); r = 1/(1+en+ep)   (via exp(-ln(en+ep+1)))
    diff = (ep-en)*r ; w_mean = diff*sc
    w_var = (1 - r - diff^2) * sc^2               (p_pos+p_neg = 1-r)
  mu = x @ w_mean.T ; s2 = (x*x) @ w_var.T ; out = mu + sqrt(s2+1e-8)*eps

Per-core shard: OS = 512 out features. x replicated; transposed + bf16/fp8-cast
on host (pure layout/marshalling) so no PE transposes are needed; theta fed
pre-transposed ([I, OS]) and the softmax weight prep runs elementwise in that
layout on ACT/DVE/GPSIMD, software-pipelined and woven against the first
chunks' matmuls (progressive-K consumption keeps the PE fed during prep).

Matmul dtypes (PE cost is output-columns x cycles/row):
  mu:   k-blocks 0..23 in bf16 (1.0 cyc/row) + k-blocks 24..31 in fp8e4
        DoubleRow (K=256/instr, 0.5 cyc/row, 4x the f32r MAC rate), joining
        the same psum accumulation group. Scales are pre-multiplied by 2^8 on
        the host (exact in bf16) so w_mean is representable in e4m3; the
        epilogue rescales mu by 2^-8.
  s2:   all-fp8e4 DoubleRow with full-width stationary [128, 2, 128].
        w_var carries a net 2^12 pre-scale (e4m3 would underflow at
        sc^2 ~ 1e-4..2.5e-3): (1-u)/16 times the 2^16-scaled sc^2; the
        epilogue sqrt() rescales by 2^-12.
        s2 is a positive-weighted dot product, so fp8 rounding noise averages
        out (~1e-3 rel) instead of accumulating; the fp8 mu tail costs
        ~1.6e-2 rel err against the 2e-2 budget.

PSUM: each accumulation group owns its own 2KB zero region (one [128, 256]
tile per (token-tile, o-half) for s2) -- two groups sharing a region loses
partials on HW (the second start re-arms the region's zero flag).

Activation-table discipline (a table load costs 1283ns): the table list given
to bacc's load-placement pass is patched (list positions preserved) so Exp+Ln
resolve to natural_log_exp_and_others and Square+Sqrt to sqrt_and_others --
2 loads total for the whole kernel.
"""
import sys

if "/opt/trn_rl_repo" not in sys.path:
    sys.path.insert(0, "/opt/trn_rl_repo")

import numpy as np

import concourse.bass as bass
import concourse.bacc as bacc
import concourse.mybir as mybir
import concourse.tile as tile
from concourse.bass_utils import run_bass_kernel_spmd
import concourse.hw_specs as hw_specs

# Steer bacc's activation-table selection: the greedy pass picks the first
# table containing each function, which splits Exp/Ln across two tables and
# re-loads (1283ns) on every alternation. Emptying the shadowing sets (while
# preserving list positions, so act_func_set_id indices stay valid for the
# neuron compiler) makes Exp+Ln resolve to natural_log_exp_and_others and
# Square+Sqrt to sqrt_and_others: two loads total instead of ~8.
_EMPTY_TABLES = {
    "exp_and_others", "softplus_and_others", "sigmoid_and_others",
    "small", "natural_log",
}
if not getattr(hw_specs, "_act_tables_patched", False):
    _orig_get_act_tables = hw_specs.get_activation_tables

    @__import__("functools").cache
    def _patched_get_act_tables(module_arch):
        tabs = _orig_get_act_tables(module_arch)
        return {name: (set() if name in _EMPTY_TABLES else s)
                for name, s in tabs.items()}

    hw_specs.get_activation_tables = _patched_get_act_tables
    hw_specs._act_tables_patched = True
    bacc.get_activation_tables = _patched_get_act_tables

N_CORES = 8
B = 4096
I = 4096
O = 4096
OS = O // N_CORES       # 512 out features per core
KB = I // 128           # 32 contraction blocks of 128
KC = KB // 2            # 16 fp8 DoubleRow pair-blocks of 256
TCH = 256               # tokens per chunk
NCH = B // TCH          # 16 chunks
VS = 4096.0             # 2^12 pre-scale for w_var before fp8 cast
F8K = 4                 # mu pair-blocks (of KC=16) done in fp8 DoubleRow
KBF = KB - 2 * F8K      # mu k-blocks done in bf16
WSC = 256.0             # scales pre-scaled by 2^8 on host so w_mean is fp8-able
F32 = mybir.dt.float32
BF16 = mybir.dt.bfloat16
F8 = mybir.dt.float8e4
PSUM = bass.MemorySpace.PSUM

_CACHE = {}


def build():
    AF = mybir.ActivationFunctionType
    OP = mybir.AluOpType
    DR = mybir.MatmulPerfMode.DoubleRow
    nc = bacc.Bacc("TRN2", target_bir_lowering=False, debug=False, num_devices=N_CORES)
    xt_d = nc.dram_tensor("xt", [I, B], BF16, kind="ExternalInput").ap()
    tn_d = nc.dram_tensor("tn", [I, OS], BF16, kind="ExternalInput").ap()
    tp_d = nc.dram_tensor("tp", [I, OS], BF16, kind="ExternalInput").ap()
    scb_d = nc.dram_tensor("scb", [KB, 128, OS], BF16, kind="ExternalInput").ap()
    xt8_d = nc.dram_tensor("xt8", [2 * F8K * 128, B], F8, kind="ExternalInput").ap()
    eps_d = nc.dram_tensor("eps", [B, OS], BF16, kind="ExternalInput").ap()
    out_d = nc.dram_tensor("out", [B, OS], F32, kind="ExternalOutput").ap()

    xt_r = xt_d.rearrange("(kb p) t -> p kb t", p=128)           # [128, 32, 4096]
    tn_r = tn_d.rearrange("(kb p) o -> p kb o", p=128)           # [128, 32, 512]
    tp_r = tp_d.rearrange("(kb p) o -> p kb o", p=128)
    scb_r = scb_d.rearrange("k p o -> p k o")                    # [128, 32, 512]
    xt8_r = xt8_d.rearrange("(kb p) t -> p kb t", p=128)         # [128, 8, 4096]
    eps_r = eps_d.rearrange("(c tt p) o -> c p tt o", p=128, tt=2)  # [16, 128, 2, 512]
    out_r = out_d.rearrange("(c tt p) o -> c p tt o", p=128, tt=2)

    with tile.TileContext(nc) as tc:
        with (
            tc.tile_pool(name="const", bufs=1) as cp,
            tc.tile_pool(name="wp", bufs=2) as wp,
            tc.tile_pool(name="xp", bufs=3) as xp,
            tc.tile_pool(name="x2p", bufs=2) as x2p,
            tc.tile_pool(name="epsp", bufs=2) as epp,
            tc.tile_pool(name="outp", bufs=2) as outp,
            tc.tile_pool(name="eip", bufs=2) as eip,
            tc.tile_pool(name="pmu", bufs=4, space=PSUM) as pmup,
            tc.tile_pool(name="pvar", bufs=4, space=PSUM) as pvp,
        ):
            WMT = cp.tile([128, KBF, OS], BF16)      # w_mean.T (bf16 k-range)
            WM8T = cp.tile([128, F8K, 2, OS], F8)    # w_mean.T fp8 DR pairs
            WVT = cp.tile([128, KC, 2, OS], F8)      # w_var.T packed DR pairs
            b1e8 = cp.tile([128, 1], F32)
            nc.vector.memset(b1e8, 1e-8)
            b1 = cp.tile([128, 1], F32)
            nc.vector.memset(b1, 1.0)

            XT = [None] * NCH     # [128, KB, TCH] bf16
            X8 = [None] * NCH     # [128, 2*F8K, TCH] f8
            X2 = [None] * NCH     # [128, KB, TCH] f8
            EPS = [None] * NCH    # [128, 2, OS] bf16
            PMU = [None] * NCH    # 2 x [128, OS] f32 psum
            PV = [None] * NCH     # [tt][h] -> [128, 256] f32 psum

            def load_xt(c):
                t = xp.tile([128, KB, TCH], BF16, tag="xt", name=f"xt{c}")
                nc.sync.dma_start(t, xt_r[:, :, TCH * c:TCH * (c + 1)])
                XT[c] = t
                load_x8(c)

            def load_x8(c):
                t = xp.tile([128, 2 * F8K, TCH], F8, tag="x8", name=f"x8{c}")
                nc.sync.dma_start(t, xt8_r[:, :, TCH * c:TCH * (c + 1)])
                X8[c] = t

            def load_eps(c):
                t = epp.tile([128, 2, OS], BF16, tag="eps", name=f"eps{c}")
                nc.sync.dma_start(t, eps_r[c])
                EPS[c] = t

            def square_x(c, act=True):
                # Square shares whatever activation table is live (it is in
                # every set), so steady-state x^2 on ACT costs no table
                # reloads against the epilogue Sqrt. During the prep window
                # ACT paces the weight prep, so chunk 0 squares on the
                # gpsimd and chunk 1 here right at the prep/steady boundary.
                t = x2p.tile([128, KB, TCH], F8, tag="x2", name=f"x2{c}")
                if act:
                    nc.scalar.square(t, XT[c])
                else:
                    nc.vector.tensor_mul(t, XT[c], XT[c])
                X2[c] = t

            def alloc_mu(c):
                PMU[c] = [pmup.tile([128, OS], F32, tag="pm", name=f"pm{c}_{tt}")
                          for tt in range(2)]

            def emit_mu(c, k_lo, k_hi):
                for k in range(k_lo, k_hi):
                    for tt in range(2):
                        nc.tensor.matmul(
                            PMU[c][tt], XT[c][:, k, 128 * tt:128 * (tt + 1)],
                            WMT[:, k, :], start=(k == 0), stop=False,
                            skip_group_check=True)

            def emit_mu8(c):
                # fp8 DoubleRow tail of the mu accumulation (k >= KBF): joins
                # the open psum group of the bf16 part. w_mean is pre-scaled
                # by 2^8 (host scb scaling), exact in both bf16 and fp8.
                for j in range(F8K):
                    for tt in range(2):
                        lhs = X8[c][:, 2 * j:2 * j + 2, 128 * tt:128 * (tt + 1)]
                        for h in range(2):
                            nc.tensor.matmul(
                                PMU[c][tt][:, 256 * h:256 * (h + 1)], lhs,
                                WM8T[:, j, :, 256 * h:256 * (h + 1)],
                                start=False, stop=(j == F8K - 1),
                                perf_mode=DR, skip_group_check=True)

            def alloc_var(b):
                # separate psum tile (= zero region) per (tt, o-half): two
                # accumulation groups must NOT share a 2KB psum zero region --
                # the second group's start_tensor_calc re-arms the whole
                # region and the first group's next accumulate gets zeroed
                # (verified on HW: sharing loses one kc-pair of sigma^2).
                PV[b] = [[pvp.tile([128, 256], F32, tag="pv", name=f"pv{b}_{tt}{h}")
                          for h in range(2)] for tt in range(2)]

            def emit_var(b, kc_lo, kc_hi, tt_major=False):
                # DoubleRow with full-width stationary [128, 2, 128]: K=256 and
                # M=128 per instruction, out [128, 256] in 128 cycles (x4 the
                # f32r MAC rate). Quadrant tiles (col=64) fail the walrus ISA
                # check in DR mode, so the stationary must span 2x128.
                # tt-major order lets each token-tile's epilogue start while
                # the other token-tile's accumulation still runs.
                order = ([(tt, kc) for tt in range(2) for kc in range(kc_lo, kc_hi)]
                         if tt_major else
                         [(tt, kc) for kc in range(kc_lo, kc_hi) for tt in range(2)])
                for tt, kc in order:
                    lhs = X2[b][:, 2 * kc:2 * kc + 2, 128 * tt:128 * (tt + 1)]
                    for h in range(2):
                        nc.tensor.matmul(
                            PV[b][tt][h], lhs,
                            WVT[:, kc, :, 256 * h:256 * (h + 1)],
                            start=(kc == 0), stop=(kc == KC - 1),
                            perf_mode=DR)

            def emit_epi(b):
                ot = outp.tile([128, 2, OS], F32, tag="out", name=f"out{b}")
                for tt in range(2):
                    for h in range(2):
                        sg = eip.tile([128, 256], BF16, tag="sg", name="sg")
                        nc.scalar.activation(sg, PV[b][tt][h], AF.Sqrt,
                                             bias=b1e8, scale=1.0 / VS)
                        pr = eip.tile([128, 256], BF16, tag="pr", name="pr")
                        nc.vector.tensor_mul(pr, sg, EPS[b][:, tt, 256 * h:256 * (h + 1)])
                        nc.vector.scalar_tensor_tensor(
                            ot[:, tt, 256 * h:256 * (h + 1)],
                            PMU[b][tt][:, 256 * h:256 * (h + 1)], 1.0 / WSC, pr,
                            op0=OP.mult, op1=OP.add)
                nc.sync.dma_start(out_r[b], ot)

            def prep_front_i(i):
                """Blocks 2i, 2i+1: dma + the two exps + S/d ([128, 2, OS])."""
                k0 = 2 * i
                sct = wp.tile([128, 2, OS], BF16, tag="sct", bufs=3, name="sct")
                nc.sync.dma_start(sct, scb_r[:, k0:k0 + 2, :])
                tn_t = wp.tile([128, 2, OS], BF16, tag="tn", name="tn")
                nc.sync.dma_start(tn_t, tn_r[:, k0:k0 + 2, :])
                tp_t = wp.tile([128, 2, OS], BF16, tag="tp", name="tp")
                nc.sync.dma_start(tp_t, tp_r[:, k0:k0 + 2, :])
                en = wp.tile([128, 2, OS], BF16, tag="en", name="en")
                nc.scalar.activation(en, tn_t, AF.Exp)
                ep = wp.tile([128, 2, OS], BF16, tag="ep", name="ep")
                nc.scalar.activation(ep, tp_t, AF.Exp)
                S = wp.tile([128, 2, OS], BF16, tag="S", name="S")
                nc.vector.tensor_add(S, en, ep)
                d = wp.tile([128, 2, OS], BF16, tag="d", bufs=3, name="d")
                nc.vector.tensor_sub(d, ep, en)
                return S, d, sct

            def prep_mid_i(S):
                """ln/exp reciprocal for iter i, emitted during iter i+1 so the
                ACT never waits on the same iter's S."""
                lnz = wp.tile([128, 2, OS], BF16, tag="lnz", name="lnz")
                nc.scalar.activation(lnz, S, AF.Ln, bias=b1)
                r = wp.tile([128, 2, OS], BF16, tag="r", name="r")
                nc.scalar.activation(r, lnz, AF.Exp, scale=-1.0)
                return r

            def prep_d_i(i, d, r, sct):
                """DVE/GPSIMD tail for blocks 2i, 2i+1; emitted one iter later
                (software pipeline) so the DVE never stalls on the iter's own
                reciprocal."""
                k0 = 2 * i
                sc2 = wp.tile([128, 2, OS], BF16, tag="sc2", name="sc2")
                nc.gpsimd.tensor_mul(sc2, sct, sct)
                diff = wp.tile([128, 2, OS], BF16, tag="diff", name="diff")
                nc.vector.tensor_mul(diff, d, r)
                if k0 < KBF:
                    nc.vector.tensor_mul(WMT[:, k0:k0 + 2, :], diff, sct)
                else:
                    wm8s = wp.tile([128, 2, OS], BF16, tag="wm8s", bufs=1, name="wm8s")
                    nc.vector.tensor_mul(wm8s, diff, sct)
                    nc.gpsimd.tensor_copy(WM8T[:, (k0 - KBF) // 2, :, :], wm8s)
                d2 = wp.tile([128, 2, OS], BF16, tag="d2", bufs=1, name="d2")
                nc.vector.tensor_mul(d2, diff, diff)
                u = wp.tile([128, 2, OS], BF16, tag="u", bufs=1, name="u")
                if i % 2 == 0:
                    nc.gpsimd.tensor_add(u, d2, r)
                else:
                    nc.vector.tensor_add(u, d2, r)
                v = wp.tile([128, 2, OS], BF16, tag="v", bufs=1, name="v")
                nc.vector.tensor_scalar(v, u, -1.0 / 16.0, 1.0 / 16.0, op0=OP.mult, op1=OP.add)
                nc.vector.tensor_mul(WVT[:, i, :, :], v, sc2)

            # ---- prep phase, woven with chunks 0/1 mu + chunk 0 var ----
            alloc_mu(0)
            alloc_mu(1)
            alloc_var(0)
            var_done = 0
            mu_done = [0, 0]
            fronts = {}
            mids = {}
            ds_done = set()

            def emit_mid(i):
                mids[i] = prep_mid_i(fronts[i][0])

            def emit_d(i):
                _, d_, sct_ = fronts.pop(i)
                prep_d_i(i, d_, mids.pop(i), sct_)
                ds_done.add(i)

            for i in range(KC):
                fronts[i] = prep_front_i(i)
                if i >= 1:
                    emit_mid(i - 1)
                if i >= 2:
                    emit_d(i - 2)
                if i == 0:
                    load_xt(0)
                if i == 2:
                    # x^2 for chunk 0 on the gpsimd (emitted after two D
                    # phases so the early u ops aren't queued behind it)
                    t0_ = x2p.tile([128, KB, TCH], F8, tag="x2", name="x2c0")
                    nc.gpsimd.tensor_mul(t0_, XT[0], XT[0])
                    X2[0] = t0_
                    load_xt(1)
                if i == 4:
                    load_eps(0)
                if i == 6:
                    load_xt(2)
                if i == 10:
                    load_eps(1)
                avail = 2 * len(ds_done)
                emit_mu(0, mu_done[0], min(avail, KBF))
                mu_done[0] = min(avail, KBF)
                hi1 = max(0, min(avail - 8, KBF))
                emit_mu(1, mu_done[1], hi1)
                mu_done[1] = hi1
                hiv = max(0, min(len(ds_done) - 2, KC))
                emit_var(0, var_done, hiv)
                var_done = hiv
            emit_mid(KC - 1)
            for i in (KC - 2, KC - 1):
                emit_d(i)
            square_x(1)
            # flush woven tails
            emit_mu(0, mu_done[0], KBF)
            emit_mu(1, mu_done[1], KBF)
            emit_mu8(0)
            emit_mu8(1)
            emit_var(0, var_done, KC)

            # ---- steady stages ----
            emit_epi(0)
            alloc_var(1)
            emit_var(1, 0, KC, tt_major=True)
            emit_epi(1)
            for c in range(2, NCH + 1):
                b = c - 1
                if c < NCH:
                    if c + 1 < NCH:
                        load_xt(c + 1)
                    load_eps(c)
                    square_x(c)
                    alloc_mu(c)
                    if b >= 2:
                        # interleave var(c-1) into mu(c)'s k-loop so the PE
                        # tail after the last mu chunk is just one epilogue
                        alloc_var(b)
                        for k in range(KBF):
                            emit_mu(c, k, k + 1)
                            if k % 2 == 1 and k // 2 < KC - F8K:
                                emit_var(b, k // 2, k // 2 + 1)
                        emit_mu8(c)
                        emit_var(b, KC - F8K, KC, tt_major=True)
                        emit_epi(b)
                    else:
                        emit_mu(c, 0, KBF)
                        emit_mu8(c)
                else:
                    alloc_var(b)
                    emit_var(b, 0, KC, tt_major=True)
                    emit_epi(b)

    nc.compile()
    return nc


def _get_nc():
    if "nc" not in _CACHE:
        _CACHE["nc"] = build()
    return _CACHE["nc"]


def kernel(x, theta_neg, theta_pos, scales_exp, eps):
    import ml_dtypes
    bf16 = ml_dtypes.bfloat16
    nc = _get_nc()
    xt = np.ascontiguousarray(np.asarray(x, np.float32).T).astype(bf16)
    f8 = ml_dtypes.float8_e4m3
    xt8 = np.ascontiguousarray(xt[128 * KBF:].astype(f8))
    eps_f = np.asarray(eps, np.float32)
    tneg = np.asarray(theta_neg, np.float32)
    tpos = np.asarray(theta_pos, np.float32)
    scal = np.asarray(scales_exp, np.float32)
    in_maps = []
    for j in range(N_CORES):
        sl = slice(OS * j, OS * (j + 1))
        sc = (scal[sl, ::128].T * WSC).astype(bf16)  # [KB, OS], pre-scaled 2^8
        scb = np.ascontiguousarray(
            np.broadcast_to(sc[:, None, :], (KB, 128, OS)))
        in_maps.append({
            "xt": xt, "xt8": xt8,
            "tn": np.ascontiguousarray(tneg[sl].T).astype(bf16),
            "tp": np.ascontiguousarray(tpos[sl].T).astype(bf16),
            "scb": scb,
            "eps": np.ascontiguousarray(eps_f[:, sl]).astype(bf16),
        })
    res = run_bass_kernel_spmd(nc, in_maps, core_ids=list(range(N_CORES)))
    return np.concatenate([res.results[j]["out"] for j in range(N_CORES)], axis=1)


# revision 47
# speedup vs baseline: 1.0021x; 1.0021x over previous
"""LRNetLinear forward on 8 Trainium2 NeuronCores (tensor-parallel over out_features).

Math (per reference):
  3-way softmax over logits [theta_neg, 0, theta_pos]:
    en = exp(tn); ep = exp(tp); r = 1/(1+en+ep)   (via exp(-ln(en+ep+1)))
    diff = (ep-en)*r ; w_mean = diff*sc
    w_var = (1 - r - diff^2) * sc^2               (p_pos+p_neg = 1-r)
  mu = x @ w_mean.T ; s2 = (x*x) @ w_var.T ; out = mu + sqrt(s2+1e-8)*eps

Per-core shard: OS = 512 out features. x replicated; transposed + bf16/fp8-cast
on host (pure layout/marshalling) so no PE transposes are needed; theta fed
pre-transposed ([I, OS]) and the softmax weight prep runs elementwise in that
layout on ACT/DVE/GPSIMD, software-pipelined and woven against the first
chunks' matmuls (progressive-K consumption keeps the PE fed during prep).

Matmul dtypes (PE cost is output-columns x cycles/row):
  mu:   k-blocks 0..23 in bf16 (1.0 cyc/row) + k-blocks 24..31 in fp8e4
        DoubleRow (K=256/instr, 0.5 cyc/row, 4x the f32r MAC rate), joining
        the same psum accumulation group. Scales are pre-multiplied by 2^8 on
        the host (exact in bf16) so w_mean is representable in e4m3; the
        epilogue rescales mu by 2^-8.
  s2:   all-fp8e4 DoubleRow with full-width stationary [128, 2, 128].
        w_var carries a net 2^12 pre-scale (e4m3 would underflow at
        sc^2 ~ 1e-4..2.5e-3): (1-u)/16 times the 2^16-scaled sc^2; the
        epilogue sqrt() rescales by 2^-12.
        s2 is a positive-weighted dot product, so fp8 rounding noise averages
        out (~1e-3 rel) instead of accumulating; the fp8 mu tail costs
        ~1.6e-2 rel err against the 2e-2 budget.

PSUM: each accumulation group owns its own 2KB zero region (one [128, 256]
tile per (token-tile, o-half) for s2) -- two groups sharing a region loses
partials on HW (the second start re-arms the region's zero flag).

Activation-table discipline (a table load costs 1283ns): the table list given
to bacc's load-placement pass is patched (list positions preserved) so Exp+Ln
resolve to natural_log_exp_and_others and Square+Sqrt to sqrt_and_others --
2 loads total for the whole kernel.
"""
import sys

if "/opt/trn_rl_repo" not in sys.path:
    sys.path.insert(0, "/opt/trn_rl_repo")

import numpy as np

import concourse.bass as bass
import concourse.bacc as bacc
import concourse.mybir as mybir
import concourse.tile as tile
from concourse.bass_utils import run_bass_kernel_spmd
import concourse.hw_specs as hw_specs

# Steer bacc's activation-table selection: the greedy pass picks the first
# table containing each function, which splits Exp/Ln across two tables and
# re-loads (1283ns) on every alternation. Emptying the shadowing sets (while
# preserving list positions, so act_func_set_id indices stay valid for the
# neuron compiler) makes Exp+Ln resolve to natural_log_exp_and_others and
# Square+Sqrt to sqrt_and_others: two loads total instead of ~8.
_EMPTY_TABLES = {
    "exp_and_others", "softplus_and_others", "sigmoid_and_others",
    "small", "natural_log",
}
if not getattr(hw_specs, "_act_tables_patched", False):
    _orig_get_act_tables = hw_specs.get_activation_tables

    @__import__("functools").cache
    def _patched_get_act_tables(module_arch):
        tabs = _orig_get_act_tables(module_arch)
        return {name: (set() if name in _EMPTY_TABLES else s)
                for name, s in tabs.items()}

    hw_specs.get_activation_tables = _patched_get_act_tables
    hw_specs._act_tables_patched = True
    bacc.get_activation_tables = _patched_get_act_tables

N_CORES = 8
B = 4096
I = 4096
O = 4096
OS = O // N_CORES       # 512 out features per core
KB = I // 128           # 32 contraction blocks of 128
KC = KB // 2            # 16 fp8 DoubleRow pair-blocks of 256
TCH = 256               # tokens per chunk
NCH = B // TCH          # 16 chunks
VS = 4096.0             # 2^12 pre-scale for w_var before fp8 cast
F8K = 4                 # mu pair-blocks (of KC=16) done in fp8 DoubleRow
KBF = KB - 2 * F8K      # mu k-blocks done in bf16
WSC = 256.0             # scales pre-scaled by 2^8 on host so w_mean is fp8-able
F32 = mybir.dt.float32
BF16 = mybir.dt.bfloat16
F8 = mybir.dt.float8e4
PSUM = bass.MemorySpace.PSUM

_CACHE = {}


def build():
    AF = mybir.ActivationFunctionType
    OP = mybir.AluOpType
    DR = mybir.MatmulPerfMode.DoubleRow
    nc = bacc.Bacc("TRN2", target_bir_lowering=False, debug=False, num_devices=N_CORES)
    xt_d = nc.dram_tensor("xt", [I, B], BF16, kind="ExternalInput").ap()
    tn_d = nc.dram_tensor("tn", [I, OS], BF16, kind="ExternalInput").ap()
    tp_d = nc.dram_tensor("tp", [I, OS], BF16, kind="ExternalInput").ap()
    scb_d = nc.dram_tensor("scb", [KB, 128, OS], BF16, kind="ExternalInput").ap()
    xt8_d = nc.dram_tensor("xt8", [2 * F8K * 128, B], F8, kind="ExternalInput").ap()
    eps_d = nc.dram_tensor("eps", [B, OS], BF16, kind="ExternalInput").ap()
    out_d = nc.dram_tensor("out", [B, OS], F32, kind="ExternalOutput").ap()

    xt_r = xt_d.rearrange("(kb p) t -> p kb t", p=128)           # [128, 32, 4096]
    tn_r = tn_d.rearrange("(kb p) o -> p kb o", p=128)           # [128, 32, 512]
    tp_r = tp_d.rearrange("(kb p) o -> p kb o", p=128)
    scb_r = scb_d.rearrange("k p o -> p k o")                    # [128, 32, 512]
    xt8_r = xt8_d.rearrange("(kb p) t -> p kb t", p=128)         # [128, 8, 4096]
    eps_r = eps_d.rearrange("(c tt p) o -> c p tt o", p=128, tt=2)  # [16, 128, 2, 512]
    out_r = out_d.rearrange("(c tt p) o -> c p tt o", p=128, tt=2)

    with tile.TileContext(nc) as tc:
        with (
            tc.tile_pool(name="const", bufs=1) as cp,
            tc.tile_pool(name="wp", bufs=2) as wp,
            tc.tile_pool(name="xp", bufs=3) as xp,
            tc.tile_pool(name="x2p", bufs=2) as x2p,
            tc.tile_pool(name="epsp", bufs=2) as epp,
            tc.tile_pool(name="outp", bufs=2) as outp,
            tc.tile_pool(name="eip", bufs=2) as eip,
            tc.tile_pool(name="pmu", bufs=4, space=PSUM) as pmup,
            tc.tile_pool(name="pvar", bufs=4, space=PSUM) as pvp,
        ):
            WMT = cp.tile([128, KBF, OS], BF16)      # w_mean.T (bf16 k-range)
            WM8T = cp.tile([128, F8K, 2, OS], F8)    # w_mean.T fp8 DR pairs
            WVT = cp.tile([128, KC, 2, OS], F8)      # w_var.T packed DR pairs
            b1e8 = cp.tile([128, 1], F32)
            nc.vector.memset(b1e8, 1e-8)
            b1 = cp.tile([128, 1], F32)
            nc.vector.memset(b1, 1.0)

            XT = [None] * NCH     # [128, KB, TCH] bf16
            X8 = [None] * NCH     # [128, 2*F8K, TCH] f8
            X2 = [None] * NCH     # [128, KB, TCH] f8
            EPS = [None] * NCH    # [128, 2, OS] bf16
            PMU = [None] * NCH    # 2 x [128, OS] f32 psum
            PV = [None] * NCH     # [tt][h] -> [128, 256] f32 psum

            def load_xt(c):
                t = xp.tile([128, KB, TCH], BF16, tag="xt", name=f"xt{c}")
                nc.sync.dma_start(t, xt_r[:, :, TCH * c:TCH * (c + 1)])
                XT[c] = t
                load_x8(c)

            def load_x8(c):
                t = xp.tile([128, 2 * F8K, TCH], F8, tag="x8", name=f"x8{c}")
                nc.sync.dma_start(t, xt8_r[:, :, TCH * c:TCH * (c + 1)])
                X8[c] = t

            def load_eps(c):
                t = epp.tile([128, 2, OS], BF16, tag="eps", name=f"eps{c}")
                nc.sync.dma_start(t, eps_r[c])
                EPS[c] = t

            def square_x(c, act=True):
                # Square shares whatever activation table is live (it is in
                # every set), so steady-state x^2 on ACT costs no table
                # reloads against the epilogue Sqrt. During the prep window
                # ACT paces the weight prep, so chunk 0 squares on the
                # gpsimd and chunk 1 here right at the prep/steady boundary.
                t = x2p.tile([128, KB, TCH], F8, tag="x2", name=f"x2{c}")
                if act:
                    nc.scalar.square(t, XT[c])
                else:
                    nc.vector.tensor_mul(t, XT[c], XT[c])
                X2[c] = t

            def alloc_mu(c):
                PMU[c] = [pmup.tile([128, OS], F32, tag="pm", name=f"pm{c}_{tt}")
                          for tt in range(2)]

            def emit_mu(c, k_lo, k_hi):
                for k in range(k_lo, k_hi):
                    for tt in range(2):
                        nc.tensor.matmul(
                            PMU[c][tt], XT[c][:, k, 128 * tt:128 * (tt + 1)],
                            WMT[:, k, :], start=(k == 0), stop=False,
                            skip_group_check=True)

            def emit_mu8(c):
                # fp8 DoubleRow tail of the mu accumulation (k >= KBF): joins
                # the open psum group of the bf16 part. w_mean is pre-scaled
                # by 2^8 (host scb scaling), exact in both bf16 and fp8.
                for j in range(F8K):
                    for tt in range(2):
                        lhs = X8[c][:, 2 * j:2 * j + 2, 128 * tt:128 * (tt + 1)]
                        for h in range(2):
                            nc.tensor.matmul(
                                PMU[c][tt][:, 256 * h:256 * (h + 1)], lhs,
                                WM8T[:, j, :, 256 * h:256 * (h + 1)],
                                start=False, stop=(j == F8K - 1),
                                perf_mode=DR, skip_group_check=True)

            def alloc_var(b):
                # separate psum tile (= zero region) per (tt, o-half): two
                # accumulation groups must NOT share a 2KB psum zero region --
                # the second group's start_tensor_calc re-arms the whole
                # region and the first group's next accumulate gets zeroed
                # (verified on HW: sharing loses one kc-pair of sigma^2).
                PV[b] = [[pvp.tile([128, 256], F32, tag="pv", name=f"pv{b}_{tt}{h}")
                          for h in range(2)] for tt in range(2)]

            def emit_var(b, kc_lo, kc_hi, tt_major=False):
                # DoubleRow with full-width stationary [128, 2, 128]: K=256 and
                # M=128 per instruction, out [128, 256] in 128 cycles (x4 the
                # f32r MAC rate). Quadrant tiles (col=64) fail the walrus ISA
                # check in DR mode, so the stationary must span 2x128.
                # tt-major order lets each token-tile's epilogue start while
                # the other token-tile's accumulation still runs.
                order = ([(tt, kc) for tt in range(2) for kc in range(kc_lo, kc_hi)]
                         if tt_major else
                         [(tt, kc) for kc in range(kc_lo, kc_hi) for tt in range(2)])
                for tt, kc in order:
                    lhs = X2[b][:, 2 * kc:2 * kc + 2, 128 * tt:128 * (tt + 1)]
                    for h in range(2):
                        nc.tensor.matmul(
                            PV[b][tt][h], lhs,
                            WVT[:, kc, :, 256 * h:256 * (h + 1)],
                            start=(kc == 0), stop=(kc == KC - 1),
                            perf_mode=DR)

            def emit_epi_tt(b, tt, ot):
                for h in range(2):
                    sg = eip.tile([128, 256], BF16, tag="sg", name="sg")
                    nc.scalar.activation(sg, PV[b][tt][h], AF.Sqrt,
                                         bias=b1e8, scale=1.0 / VS)
                    pr = eip.tile([128, 256], BF16, tag="pr", name="pr")
                    nc.vector.tensor_mul(pr, sg, EPS[b][:, tt, 256 * h:256 * (h + 1)])
                    nc.vector.scalar_tensor_tensor(
                        ot[:, tt, 256 * h:256 * (h + 1)],
                        PMU[b][tt][:, 256 * h:256 * (h + 1)], 1.0 / WSC, pr,
                        op0=OP.mult, op1=OP.add)
                nc.sync.dma_start(out_r[b][:, tt, :], ot[:, tt, :])

            def emit_epi(b):
                ot = outp.tile([128, 2, OS], F32, tag="out", name=f"out{b}")
                for tt in range(2):
                    for h in range(2):
                        sg = eip.tile([128, 256], BF16, tag="sg", name="sg")
                        nc.scalar.activation(sg, PV[b][tt][h], AF.Sqrt,
                                             bias=b1e8, scale=1.0 / VS)
                        pr = eip.tile([128, 256], BF16, tag="pr", name="pr")
                        nc.vector.tensor_mul(pr, sg, EPS[b][:, tt, 256 * h:256 * (h + 1)])
                        nc.vector.scalar_tensor_tensor(
                            ot[:, tt, 256 * h:256 * (h + 1)],
                            PMU[b][tt][:, 256 * h:256 * (h + 1)], 1.0 / WSC, pr,
                            op0=OP.mult, op1=OP.add)
                nc.sync.dma_start(out_r[b], ot)

            def prep_front_i(i):
                """Blocks 2i, 2i+1: dma + the two exps + S/d ([128, 2, OS])."""
                k0 = 2 * i
                tn_t = wp.tile([128, 2, OS], BF16, tag="tn", name="tn")
                nc.sync.dma_start(tn_t, tn_r[:, k0:k0 + 2, :])
                tp_t = wp.tile([128, 2, OS], BF16, tag="tp", name="tp")
                nc.sync.dma_start(tp_t, tp_r[:, k0:k0 + 2, :])
                sct = wp.tile([128, 2, OS], BF16, tag="sct", bufs=3, name="sct")
                nc.sync.dma_start(sct, scb_r[:, k0:k0 + 2, :])
                en = wp.tile([128, 2, OS], BF16, tag="en", name="en")
                nc.scalar.activation(en, tn_t, AF.Exp)
                ep = wp.tile([128, 2, OS], BF16, tag="ep", name="ep")
                nc.scalar.activation(ep, tp_t, AF.Exp)
                S = wp.tile([128, 2, OS], BF16, tag="S", name="S")
                nc.vector.tensor_add(S, en, ep)
                d = wp.tile([128, 2, OS], BF16, tag="d", bufs=3, name="d")
                nc.vector.tensor_sub(d, ep, en)
                return S, d, sct

            def prep_mid_i(S):
                """ln/exp reciprocal for iter i, emitted during iter i+1 so the
                ACT never waits on the same iter's S."""
                lnz = wp.tile([128, 2, OS], BF16, tag="lnz", name="lnz")
                nc.scalar.activation(lnz, S, AF.Ln, bias=b1)
                r = wp.tile([128, 2, OS], BF16, tag="r", name="r")
                nc.scalar.activation(r, lnz, AF.Exp, scale=-1.0)
                return r

            def prep_d_i(i, d, r, sct):
                """DVE/GPSIMD tail for blocks 2i, 2i+1; emitted one iter later
                (software pipeline) so the DVE never stalls on the iter's own
                reciprocal."""
                k0 = 2 * i
                sc2 = wp.tile([128, 2, OS], BF16, tag="sc2", name="sc2")
                nc.gpsimd.tensor_mul(sc2, sct, sct)
                diff = wp.tile([128, 2, OS], BF16, tag="diff", name="diff")
                nc.vector.tensor_mul(diff, d, r)
                if k0 < KBF:
                    nc.vector.tensor_mul(WMT[:, k0:k0 + 2, :], diff, sct)
                else:
                    wm8s = wp.tile([128, 2, OS], BF16, tag="wm8s", bufs=1, name="wm8s")
                    nc.vector.tensor_mul(wm8s, diff, sct)
                    nc.gpsimd.tensor_copy(WM8T[:, (k0 - KBF) // 2, :, :], wm8s)
                d2 = wp.tile([128, 2, OS], BF16, tag="d2", name="d2")
                nc.vector.tensor_mul(d2, diff, diff)
                u = wp.tile([128, 2, OS], BF16, tag="u", name="u")
                if i % 2 == 0:
                    nc.gpsimd.tensor_add(u, d2, r)
                else:
                    nc.vector.tensor_add(u, d2, r)
                v = wp.tile([128, 2, OS], BF16, tag="v", name="v")
                nc.vector.tensor_scalar(v, u, -1.0 / 16.0, 1.0 / 16.0, op0=OP.mult, op1=OP.add)
                nc.vector.tensor_mul(WVT[:, i, :, :], v, sc2)

            # ---- prep phase, woven with chunks 0/1 mu + chunk 0 var ----
            alloc_mu(0)
            alloc_mu(1)
            alloc_var(0)
            var_done = 0
            mu_done = [0, 0]
            fronts = {}
            mids = {}
            ds_done = set()

            def emit_mid(i):
                mids[i] = prep_mid_i(fronts[i][0])

            def emit_d(i):
                _, d_, sct_ = fronts.pop(i)
                prep_d_i(i, d_, mids.pop(i), sct_)
                ds_done.add(i)

            for i in range(KC):
                fronts[i] = prep_front_i(i)
                if i >= 1:
                    emit_mid(i - 1)
                if i >= 2:
                    emit_d(i - 2)
                if i == 0:
                    load_xt(0)
                if i == 2:
                    # x^2 for chunk 0 on the gpsimd (emitted after two D
                    # phases so the early u ops aren't queued behind it)
                    t0_ = x2p.tile([128, KB, TCH], F8, tag="x2", name="x2c0")
                    nc.gpsimd.tensor_mul(t0_, XT[0], XT[0])
                    X2[0] = t0_
                    load_xt(1)
                if i == 4:
                    load_eps(0)
                if i == 6:
                    load_xt(2)
                if i == 10:
                    load_eps(1)
                avail = 2 * len(ds_done)
                emit_mu(0, mu_done[0], min(avail, KBF))
                mu_done[0] = min(avail, KBF)
                hi1 = max(0, min(avail - 8, KBF))
                emit_mu(1, mu_done[1], hi1)
                mu_done[1] = hi1
                hiv = max(0, min(len(ds_done) - 2, KC))
                emit_var(0, var_done, hiv)
                var_done = hiv
            emit_mid(KC - 1)
            for i in (KC - 2, KC - 1):
                emit_d(i)
            square_x(1)
            # flush woven tails
            emit_mu(0, mu_done[0], KBF)
            emit_mu(1, mu_done[1], KBF)
            emit_mu8(0)
            emit_mu8(1)
            emit_var(0, var_done, KC)

            # ---- steady stages ----
            emit_epi(0)
            alloc_var(1)
            emit_var(1, 0, KC, tt_major=True)
            emit_epi(1)
            for c in range(2, NCH + 1):
                b = c - 1
                if c < NCH:
                    if c + 1 < NCH:
                        load_xt(c + 1)
                    load_eps(c)
                    square_x(c)
                    alloc_mu(c)
                    if b >= 2:
                        # interleave var(c-1) into mu(c)'s k-loop so the PE
                        # tail after the last mu chunk is just one epilogue
                        alloc_var(b)
                        for k in range(KBF):
                            emit_mu(c, k, k + 1)
                            if k % 2 == 1 and k // 2 < KC - F8K:
                                emit_var(b, k // 2, k // 2 + 1)
                        emit_mu8(c)
                        emit_var(b, KC - F8K, KC, tt_major=True)
                        emit_epi(b)
                    else:
                        emit_mu(c, 0, KBF)
                        emit_mu8(c)
                else:
                    # last chunk: per-tt var + epilogue so the final epilogue
                    # chain overlaps the other token-tile's accumulation
                    alloc_var(b)
                    ot = outp.tile([128, 2, OS], F32, tag="out", name=f"out{b}")
                    for tt in range(2):
                        for kc in range(KC):
                            lhs = X2[b][:, 2 * kc:2 * kc + 2, 128 * tt:128 * (tt + 1)]
                            for h in range(2):
                                nc.tensor.matmul(
                                    PV[b][tt][h], lhs,
                                    WVT[:, kc, :, 256 * h:256 * (h + 1)],
                                    start=(kc == 0), stop=(kc == KC - 1),
                                    perf_mode=DR)
                        emit_epi_tt(b, tt, ot)

    nc.compile()
    return nc


def _get_nc():
    if "nc" not in _CACHE:
        _CACHE["nc"] = build()
    return _CACHE["nc"]


def kernel(x, theta_neg, theta_pos, scales_exp, eps):
    import ml_dtypes
    bf16 = ml_dtypes.bfloat16
    nc = _get_nc()
    xt = np.ascontiguousarray(np.asarray(x, np.float32).T).astype(bf16)
    f8 = ml_dtypes.float8_e4m3
    xt8 = np.ascontiguousarray(xt[128 * KBF:].astype(f8))
    eps_f = np.asarray(eps, np.float32)
    tneg = np.asarray(theta_neg, np.float32)
    tpos = np.asarray(theta_pos, np.float32)
    scal = np.asarray(scales_exp, np.float32)
    in_maps = []
    for j in range(N_CORES):
        sl = slice(OS * j, OS * (j + 1))
        sc = (scal[sl, ::128].T * WSC).astype(bf16)  # [KB, OS], pre-scaled 2^8
        scb = np.ascontiguousarray(
            np.broadcast_to(sc[:, None, :], (KB, 128, OS)))
        in_maps.append({
            "xt": xt, "xt8": xt8,
            "tn": np.ascontiguousarray(tneg[sl].T).astype(bf16),
            "tp": np.ascontiguousarray(tpos[sl].T).astype(bf16),
            "scb": scb,
            "eps": np.ascontiguousarray(eps_f[:, sl]).astype(bf16),
        })
    res = run_bass_kernel_spmd(nc, in_maps, core_ids=list(range(N_CORES)))
    return np.concatenate([res.results[j]["out"] for j in range(N_CORES)], axis=1)


# revision 50
# speedup vs baseline: 1.0382x; 1.0360x over previous
"""LRNetLinear forward on 8 Trainium2 NeuronCores (tensor-parallel over out_features).

Math (per reference):
  3-way softmax over logits [theta_neg, 0, theta_pos]:
    en = exp(tn); ep = exp(tp); r = 1/(1+en+ep)   (via exp(-ln(en+ep+1)))
    diff = (ep-en)*r ; w_mean = diff*sc
    w_var = (1 - r - diff^2) * sc^2               (p_pos+p_neg = 1-r)
  mu = x @ w_mean.T ; s2 = (x*x) @ w_var.T ; out = mu + sqrt(s2+1e-8)*eps

Per-core shard: OS = 512 out features. x replicated; transposed + bf16/fp8-cast
on host (pure layout/marshalling) so no PE transposes are needed; theta fed
pre-transposed ([I, OS]) and the softmax weight prep runs elementwise in that
layout on ACT/DVE/GPSIMD, software-pipelined and woven against the first
chunks' matmuls (progressive-K consumption keeps the PE fed during prep).

Matmul dtypes (PE cost is output-columns x cycles/row):
  mu:   k-blocks 0..23 in bf16 (1.0 cyc/row) + k-blocks 24..31 in fp8e4
        DoubleRow (K=256/instr, 0.5 cyc/row, 4x the f32r MAC rate), joining
        the same psum accumulation group. Scales are pre-multiplied by 2^8 on
        the host (exact in bf16) so w_mean is representable in e4m3; the
        epilogue rescales mu by 2^-8.
  s2:   all-fp8e4 DoubleRow with full-width stationary [128, 2, 128].
        w_var carries a net 2^12 pre-scale (e4m3 would underflow at
        sc^2 ~ 1e-4..2.5e-3): (1-u)/16 times the 2^16-scaled sc^2; the
        epilogue sqrt() rescales by 2^-12.
        s2 is a positive-weighted dot product, so fp8 rounding noise averages
        out (~1e-3 rel) instead of accumulating; the fp8 mu tail costs
        ~1.6e-2 rel err against the 2e-2 budget.

PSUM: each accumulation group owns its own 2KB zero region (one [128, 256]
tile per (token-tile, o-half) for s2) -- two groups sharing a region loses
partials on HW (the second start re-arms the region's zero flag).

Activation-table discipline (a table load costs 1283ns): the table list given
to bacc's load-placement pass is patched (list positions preserved) so Exp+Ln
resolve to natural_log_exp_and_others and Square+Sqrt to sqrt_and_others --
2 loads total for the whole kernel.
"""
import sys

if "/opt/trn_rl_repo" not in sys.path:
    sys.path.insert(0, "/opt/trn_rl_repo")

import numpy as np

import concourse.bass as bass
import concourse.bacc as bacc
import concourse.mybir as mybir
import concourse.tile as tile
from concourse.bass_utils import run_bass_kernel_spmd
import concourse.hw_specs as hw_specs

# Steer bacc's activation-table selection: the greedy pass picks the first
# table containing each function, which splits Exp/Ln across two tables and
# re-loads (1283ns) on every alternation. Emptying the shadowing sets (while
# preserving list positions, so act_func_set_id indices stay valid for the
# neuron compiler) makes Exp+Ln resolve to natural_log_exp_and_others and
# Square+Sqrt to sqrt_and_others: two loads total instead of ~8.
_EMPTY_TABLES = {
    "exp_and_others", "softplus_and_others", "sigmoid_and_others",
    "small", "natural_log",
}
if not getattr(hw_specs, "_act_tables_patched", False):
    _orig_get_act_tables = hw_specs.get_activation_tables

    @__import__("functools").cache
    def _patched_get_act_tables(module_arch):
        tabs = _orig_get_act_tables(module_arch)
        return {name: (set() if name in _EMPTY_TABLES else s)
                for name, s in tabs.items()}

    hw_specs.get_activation_tables = _patched_get_act_tables
    hw_specs._act_tables_patched = True
    bacc.get_activation_tables = _patched_get_act_tables

N_CORES = 8
B = 4096
I = 4096
O = 4096
OS = O // N_CORES       # 512 out features per core
KB = I // 128           # 32 contraction blocks of 128
KC = KB // 2            # 16 fp8 DoubleRow pair-blocks of 256
TCH = 256               # tokens per chunk
NCH = B // TCH          # 16 chunks
VS = 4096.0             # 2^12 pre-scale for w_var before fp8 cast
F8K = 4                 # mu pair-blocks (of KC=16) done in fp8 DoubleRow
KBF = KB - 2 * F8K      # mu k-blocks done in bf16
WSC = 256.0             # scales pre-scaled by 2^8 on host so w_mean is fp8-able
F32 = mybir.dt.float32
BF16 = mybir.dt.bfloat16
F8 = mybir.dt.float8e4
PSUM = bass.MemorySpace.PSUM

_CACHE = {}


def build():
    AF = mybir.ActivationFunctionType
    OP = mybir.AluOpType
    DR = mybir.MatmulPerfMode.DoubleRow
    nc = bacc.Bacc("TRN2", target_bir_lowering=False, debug=False, num_devices=N_CORES)
    xt_d = nc.dram_tensor("xt", [I, B], BF16, kind="ExternalInput").ap()
    tn_d = nc.dram_tensor("tn", [I, OS], BF16, kind="ExternalInput").ap()
    tp_d = nc.dram_tensor("tp", [I, OS], BF16, kind="ExternalInput").ap()
    scb_d = nc.dram_tensor("scb", [KB, 128, OS], BF16, kind="ExternalInput").ap()
    xt8_d = nc.dram_tensor("xt8", [2 * F8K * 128, B], F8, kind="ExternalInput").ap()
    eps_d = nc.dram_tensor("eps", [B, OS], BF16, kind="ExternalInput").ap()
    out_d = nc.dram_tensor("out", [B, OS], F32, kind="ExternalOutput").ap()

    xt_r = xt_d.rearrange("(kb p) t -> p kb t", p=128)           # [128, 32, 4096]
    tn_r = tn_d.rearrange("(kb p) o -> p kb o", p=128)           # [128, 32, 512]
    tp_r = tp_d.rearrange("(kb p) o -> p kb o", p=128)
    scb_r = scb_d.rearrange("k p o -> p k o")                    # [128, 32, 512]
    xt8_r = xt8_d.rearrange("(kb p) t -> p kb t", p=128)         # [128, 8, 4096]
    eps_r = eps_d.rearrange("(c tt p) o -> c p tt o", p=128, tt=2)  # [16, 128, 2, 512]
    out_r = out_d.rearrange("(c tt p) o -> c p tt o", p=128, tt=2)

    with tile.TileContext(nc) as tc:
        with (
            tc.tile_pool(name="const", bufs=1) as cp,
            tc.tile_pool(name="wp", bufs=2) as wp,
            tc.tile_pool(name="xp", bufs=3) as xp,
            tc.tile_pool(name="x2p", bufs=2) as x2p,
            tc.tile_pool(name="epsp", bufs=2) as epp,
            tc.tile_pool(name="outp", bufs=2) as outp,
            tc.tile_pool(name="eip", bufs=2) as eip,
            tc.tile_pool(name="pmu", bufs=4, space=PSUM) as pmup,
            tc.tile_pool(name="pvar", bufs=4, space=PSUM) as pvp,
        ):
            WMT = cp.tile([128, KBF, OS], BF16)      # w_mean.T (bf16 k-range)
            WM8T = cp.tile([128, F8K, 2, OS], F8)    # w_mean.T fp8 DR pairs
            WVT = cp.tile([128, KC, 2, OS], F8)      # w_var.T packed DR pairs
            b1e8 = cp.tile([128, 1], F32)
            nc.vector.memset(b1e8, 1e-8)
            b1 = cp.tile([128, 1], F32)
            nc.vector.memset(b1, 1.0)

            XT = [None] * NCH     # [128, KB, TCH] bf16
            X8 = [None] * NCH     # [128, 2*F8K, TCH] f8
            X2 = [None] * NCH     # [128, KB, TCH] f8
            EPS = [None] * NCH    # [128, 2, OS] bf16
            PMU = [None] * NCH    # 2 x [128, OS] f32 psum
            PV = [None] * NCH     # [tt][h] -> [128, 256] f32 psum

            def load_xt(c, half=None):
                # half=0/1 splits the 5.8us transfer so it doesn't stall the
                # prep theta-DMA stream for a whole iteration
                if half in (None, 0):
                    t = xp.tile([128, KB, TCH], BF16, tag="xt", name=f"xt{c}")
                    XT[c] = t
                lo, hi = {None: (0, KB), 0: (0, KB // 2), 1: (KB // 2, KB)}[half]
                nc.sync.dma_start(XT[c][:, lo:hi, :],
                                  xt_r[:, lo:hi, TCH * c:TCH * (c + 1)])
                if half in (None, 1):
                    load_x8(c)

            def load_x8(c):
                t = xp.tile([128, 2 * F8K, TCH], F8, tag="x8", name=f"x8{c}")
                nc.sync.dma_start(t, xt8_r[:, :, TCH * c:TCH * (c + 1)])
                X8[c] = t

            def load_eps(c):
                t = epp.tile([128, 2, OS], BF16, tag="eps", name=f"eps{c}")
                nc.sync.dma_start(t, eps_r[c])
                EPS[c] = t

            def square_x(c, act=True):
                # Square shares whatever activation table is live (it is in
                # every set), so steady-state x^2 on ACT costs no table
                # reloads against the epilogue Sqrt. During the prep window
                # ACT paces the weight prep, so chunk 0 squares on the
                # gpsimd and chunk 1 here right at the prep/steady boundary.
                t = x2p.tile([128, KB, TCH], F8, tag="x2", name=f"x2{c}")
                if act:
                    nc.scalar.square(t, XT[c])
                else:
                    nc.vector.tensor_mul(t, XT[c], XT[c])
                X2[c] = t

            def alloc_mu(c):
                PMU[c] = [pmup.tile([128, OS], F32, tag="pm", name=f"pm{c}_{tt}")
                          for tt in range(2)]

            def emit_mu(c, k_lo, k_hi):
                for k in range(k_lo, k_hi):
                    for tt in range(2):
                        nc.tensor.matmul(
                            PMU[c][tt], XT[c][:, k, 128 * tt:128 * (tt + 1)],
                            WMT[:, k, :], start=(k == 0), stop=False,
                            skip_group_check=True)

            def emit_mu8(c):
                # fp8 DoubleRow tail of the mu accumulation (k >= KBF): joins
                # the open psum group of the bf16 part. w_mean is pre-scaled
                # by 2^8 (host scb scaling), exact in both bf16 and fp8.
                for j in range(F8K):
                    for tt in range(2):
                        lhs = X8[c][:, 2 * j:2 * j + 2, 128 * tt:128 * (tt + 1)]
                        for h in range(2):
                            nc.tensor.matmul(
                                PMU[c][tt][:, 256 * h:256 * (h + 1)], lhs,
                                WM8T[:, j, :, 256 * h:256 * (h + 1)],
                                start=False, stop=(j == F8K - 1),
                                perf_mode=DR, skip_group_check=True)

            def alloc_var(b):
                # separate psum tile (= zero region) per (tt, o-half): two
                # accumulation groups must NOT share a 2KB psum zero region --
                # the second group's start_tensor_calc re-arms the whole
                # region and the first group's next accumulate gets zeroed
                # (verified on HW: sharing loses one kc-pair of sigma^2).
                PV[b] = [[pvp.tile([128, 256], F32, tag="pv", name=f"pv{b}_{tt}{h}")
                          for h in range(2)] for tt in range(2)]

            def emit_var(b, kc_lo, kc_hi, tt_major=False):
                # DoubleRow with full-width stationary [128, 2, 128]: K=256 and
                # M=128 per instruction, out [128, 256] in 128 cycles (x4 the
                # f32r MAC rate). Quadrant tiles (col=64) fail the walrus ISA
                # check in DR mode, so the stationary must span 2x128.
                # tt-major order lets each token-tile's epilogue start while
                # the other token-tile's accumulation still runs.
                order = ([(tt, kc) for tt in range(2) for kc in range(kc_lo, kc_hi)]
                         if tt_major else
                         [(tt, kc) for kc in range(kc_lo, kc_hi) for tt in range(2)])
                for tt, kc in order:
                    lhs = X2[b][:, 2 * kc:2 * kc + 2, 128 * tt:128 * (tt + 1)]
                    for h in range(2):
                        nc.tensor.matmul(
                            PV[b][tt][h], lhs,
                            WVT[:, kc, :, 256 * h:256 * (h + 1)],
                            start=(kc == 0), stop=(kc == KC - 1),
                            perf_mode=DR)

            def emit_epi_tt(b, tt, ot):
                for h in range(2):
                    sg = eip.tile([128, 256], BF16, tag="sg", name="sg")
                    nc.scalar.activation(sg, PV[b][tt][h], AF.Sqrt,
                                         bias=b1e8, scale=1.0 / VS)
                    pr = eip.tile([128, 256], BF16, tag="pr", name="pr")
                    nc.vector.tensor_mul(pr, sg, EPS[b][:, tt, 256 * h:256 * (h + 1)])
                    nc.vector.scalar_tensor_tensor(
                        ot[:, tt, 256 * h:256 * (h + 1)],
                        PMU[b][tt][:, 256 * h:256 * (h + 1)], 1.0 / WSC, pr,
                        op0=OP.mult, op1=OP.add)
                nc.sync.dma_start(out_r[b][:, tt, :], ot[:, tt, :])

            def emit_epi(b):
                ot = outp.tile([128, 2, OS], F32, tag="out", name=f"out{b}")
                for tt in range(2):
                    for h in range(2):
                        sg = eip.tile([128, 256], BF16, tag="sg", name="sg")
                        nc.scalar.activation(sg, PV[b][tt][h], AF.Sqrt,
                                             bias=b1e8, scale=1.0 / VS)
                        pr = eip.tile([128, 256], BF16, tag="pr", name="pr")
                        nc.vector.tensor_mul(pr, sg, EPS[b][:, tt, 256 * h:256 * (h + 1)])
                        nc.vector.scalar_tensor_tensor(
                            ot[:, tt, 256 * h:256 * (h + 1)],
                            PMU[b][tt][:, 256 * h:256 * (h + 1)], 1.0 / WSC, pr,
                            op0=OP.mult, op1=OP.add)
                nc.sync.dma_start(out_r[b], ot)

            def prep_front_i(i):
                """Blocks 2i, 2i+1: dma + the two exps + S/d ([128, 2, OS])."""
                k0 = 2 * i
                tn_t = wp.tile([128, 2, OS], BF16, tag="tn", name="tn")
                nc.sync.dma_start(tn_t, tn_r[:, k0:k0 + 2, :])
                tp_t = wp.tile([128, 2, OS], BF16, tag="tp", name="tp")
                nc.sync.dma_start(tp_t, tp_r[:, k0:k0 + 2, :])
                sct = wp.tile([128, 2, OS], BF16, tag="sct", bufs=3, name="sct")
                nc.sync.dma_start(sct, scb_r[:, k0:k0 + 2, :])
                en = wp.tile([128, 2, OS], BF16, tag="en", name="en")
                nc.scalar.activation(en, tn_t, AF.Exp)
                ep = wp.tile([128, 2, OS], BF16, tag="ep", name="ep")
                nc.scalar.activation(ep, tp_t, AF.Exp)
                S = wp.tile([128, 2, OS], BF16, tag="S", name="S")
                nc.vector.tensor_add(S, en, ep)
                d = wp.tile([128, 2, OS], BF16, tag="d", bufs=3, name="d")
                nc.vector.tensor_sub(d, ep, en)
                return S, d, sct

            def prep_mid_i(S):
                """ln/exp reciprocal for iter i, emitted during iter i+1 so the
                ACT never waits on the same iter's S."""
                lnz = wp.tile([128, 2, OS], BF16, tag="lnz", name="lnz")
                nc.scalar.activation(lnz, S, AF.Ln, bias=b1)
                r = wp.tile([128, 2, OS], BF16, tag="r", name="r")
                nc.scalar.activation(r, lnz, AF.Exp, scale=-1.0)
                return r

            def prep_d_i(i, d, r, sct):
                """DVE/GPSIMD tail for blocks 2i, 2i+1; emitted one iter later
                (software pipeline) so the DVE never stalls on the iter's own
                reciprocal."""
                k0 = 2 * i
                sc2 = wp.tile([128, 2, OS], BF16, tag="sc2", name="sc2")
                nc.gpsimd.tensor_mul(sc2, sct, sct)
                diff = wp.tile([128, 2, OS], BF16, tag="diff", name="diff")
                nc.vector.tensor_mul(diff, d, r)
                if k0 < KBF:
                    nc.vector.tensor_mul(WMT[:, k0:k0 + 2, :], diff, sct)
                else:
                    wm8s = wp.tile([128, 2, OS], BF16, tag="wm8s", bufs=1, name="wm8s")
                    nc.vector.tensor_mul(wm8s, diff, sct)
                    nc.gpsimd.tensor_copy(WM8T[:, (k0 - KBF) // 2, :, :], wm8s)
                d2 = wp.tile([128, 2, OS], BF16, tag="d2", name="d2")
                nc.vector.tensor_mul(d2, diff, diff)
                u = wp.tile([128, 2, OS], BF16, tag="u", name="u")
                nc.vector.tensor_add(u, d2, r)
                v = wp.tile([128, 2, OS], BF16, tag="v", name="v")
                nc.vector.tensor_scalar(v, u, -1.0 / 16.0, 1.0 / 16.0, op0=OP.mult, op1=OP.add)
                nc.vector.tensor_mul(WVT[:, i, :, :], v, sc2)

            # ---- prep phase, woven with chunks 0/1 mu + chunk 0 var ----
            alloc_mu(0)
            alloc_mu(1)
            alloc_var(0)
            var_done = 0
            mu_done = [0, 0]
            fronts = {}
            mids = {}
            ds_done = set()

            def emit_mid(i):
                mids[i] = prep_mid_i(fronts[i][0])

            def emit_d(i):
                _, d_, sct_ = fronts.pop(i)
                prep_d_i(i, d_, mids.pop(i), sct_)
                ds_done.add(i)

            for i in range(KC):
                fronts[i] = prep_front_i(i)
                if i >= 1:
                    emit_mid(i - 1)
                if i >= 2:
                    emit_d(i - 2)
                if i == 0:
                    load_xt(0)
                    X2[0] = x2p.tile([128, KB, TCH], F8, tag="x2", name="x2c0")
                if i == 1:
                    load_xt(1, half=0)
                if i in (2, 4, 6, 8):
                    # x^2 for chunk 0 on the gpsimd in four parts: one 16us
                    # instruction would head-of-line block sc2(i) and stall
                    # the DVE's wv chain for the whole prep start
                    q4 = (i - 2) // 2
                    nc.gpsimd.tensor_mul(X2[0][:, 8 * q4:8 * (q4 + 1), :],
                                         XT[0][:, 8 * q4:8 * (q4 + 1), :],
                                         XT[0][:, 8 * q4:8 * (q4 + 1), :])
                if i == 3:
                    load_xt(1, half=1)
                if i == 4:
                    load_eps(0)
                if i == 6:
                    load_xt(2, half=0)
                if i == 8:
                    load_xt(2, half=1)
                if i == 10:
                    load_eps(1)
                avail = 2 * len(ds_done)
                emit_mu(0, mu_done[0], min(avail, KBF))
                mu_done[0] = min(avail, KBF)
                hi1 = max(0, min(avail - 8, KBF))
                emit_mu(1, mu_done[1], hi1)
                mu_done[1] = hi1
                hiv = max(0, min(len(ds_done) - 2, KC))
                emit_var(0, var_done, hiv)
                var_done = hiv
            emit_mid(KC - 1)
            for i in (KC - 2, KC - 1):
                emit_d(i)
            square_x(1)
            # flush woven tails
            emit_mu(0, mu_done[0], KBF)
            emit_mu(1, mu_done[1], KBF)
            emit_mu8(0)
            emit_mu8(1)
            emit_var(0, var_done, KC)

            # ---- steady stages ----
            emit_epi(0)
            alloc_var(1)
            emit_var(1, 0, KC, tt_major=True)
            emit_epi(1)
            for c in range(2, NCH + 1):
                b = c - 1
                if c < NCH:
                    if c + 1 < NCH:
                        load_xt(c + 1)
                    load_eps(c)
                    square_x(c)
                    alloc_mu(c)
                    if b >= 2:
                        # interleave var(c-1) into mu(c)'s k-loop so the PE
                        # tail after the last mu chunk is just one epilogue
                        alloc_var(b)
                        for k in range(KBF):
                            emit_mu(c, k, k + 1)
                            if k % 2 == 1 and k // 2 < KC - F8K:
                                emit_var(b, k // 2, k // 2 + 1)
                        emit_mu8(c)
                        emit_var(b, KC - F8K, KC, tt_major=True)
                        emit_epi(b)
                    else:
                        emit_mu(c, 0, KBF)
                        emit_mu8(c)
                else:
                    # last chunk: per-tt var + epilogue so the final epilogue
                    # chain overlaps the other token-tile's accumulation
                    alloc_var(b)
                    ot = outp.tile([128, 2, OS], F32, tag="out", name=f"out{b}")
                    for tt in range(2):
                        for kc in range(KC):
                            lhs = X2[b][:, 2 * kc:2 * kc + 2, 128 * tt:128 * (tt + 1)]
                            for h in range(2):
                                nc.tensor.matmul(
                                    PV[b][tt][h], lhs,
                                    WVT[:, kc, :, 256 * h:256 * (h + 1)],
                                    start=(kc == 0), stop=(kc == KC - 1),
                                    perf_mode=DR)
                        emit_epi_tt(b, tt, ot)

    nc.compile()
    return nc


def _get_nc():
    if "nc" not in _CACHE:
        _CACHE["nc"] = build()
    return _CACHE["nc"]


def kernel(x, theta_neg, theta_pos, scales_exp, eps):
    import ml_dtypes
    bf16 = ml_dtypes.bfloat16
    nc = _get_nc()
    xt = np.ascontiguousarray(np.asarray(x, np.float32).T).astype(bf16)
    f8 = ml_dtypes.float8_e4m3
    xt8 = np.ascontiguousarray(xt[128 * KBF:].astype(f8))
    eps_f = np.asarray(eps, np.float32)
    tneg = np.asarray(theta_neg, np.float32)
    tpos = np.asarray(theta_pos, np.float32)
    scal = np.asarray(scales_exp, np.float32)
    in_maps = []
    for j in range(N_CORES):
        sl = slice(OS * j, OS * (j + 1))
        sc = (scal[sl, ::128].T * WSC).astype(bf16)  # [KB, OS], pre-scaled 2^8
        scb = np.ascontiguousarray(
            np.broadcast_to(sc[:, None, :], (KB, 128, OS)))
        in_maps.append({
            "xt": xt, "xt8": xt8,
            "tn": np.ascontiguousarray(tneg[sl].T).astype(bf16),
            "tp": np.ascontiguousarray(tpos[sl].T).astype(bf16),
            "scb": scb,
            "eps": np.ascontiguousarray(eps_f[:, sl]).astype(bf16),
        })
    res = run_bass_kernel_spmd(nc, in_maps, core_ids=list(range(N_CORES)))
    return np.concatenate([res.results[j]["out"] for j in range(N_CORES)], axis=1)
